# revision 1
# baseline (speedup 1.0000x reference)
"""MHSA Trainium2 kernel: 8-core batch(2) x head-quad(4) sharding.

Reference: x[2,2048,1024] @ w_qkv.T -> per-head attention -> @ w_out.T + b.
Core c = 4*g + j handles batch g, heads 4j..4j+3. It computes Q^T/K^T/V for
its heads, S^T = K Q^T per head (softmax denominator via a ones-column
appended to V), exp on ACT straight off PSUM, O^T = V_aug^T expS^T, per-head
normalization (ones-matmul broadcast of 1/rowsum), and a partial output
projection over its 256 inner dims. Host sums the 4 partials per batch and
adds the bias. All matmuls bf16, accumulation fp32.
"""
import numpy as np
import ml_dtypes

HEADS = 16
HEAD_DIM = 64
TOKEN_DIM = 1024
INNER = HEADS * HEAD_DIM
B = 2
N = 2048
HPC = 4            # heads per core
GROUPS = 2         # batches
CORES = 8

_cache = {}


def _build():
    import concourse.bass as bass
    import concourse.mybir as mybir
    from concourse.tile import TileContext

    F32 = mybir.dt.float32
    BF16 = mybir.dt.bfloat16
    AF = mybir.ActivationFunctionType

    from concourse.vector_clock import ScopedClock

    class TC(TileContext):
        # this walrus build allows only ONE sync wait per instruction; split
        # the kernel-tail drain's waits into standalone wait_ge instructions
        def _drain_and_barrier(self, tick_clock, wait_clock):
            any_sem = next(iter(self.sems.allocated().values()))
            tmp = self.nc.sync.wait_ge(any_sem, 0)
            wait_clock.add_sem_waits(
                tmp.ins, ScopedClock({None: tick_clock.global_clock})
            )
            waits = list(tmp.ins.sync_info.on_wait)
            try:
                tmp.ins.sync_info.on_wait.clear()
            except Exception:
                import concourse.mybir as _mybir
                tmp.ins.sync_info = _mybir.SyncInfo(
                    on_wait=[], on_update=list(tmp.ins.sync_info.on_update)
                )
            sem_by_name = {}
            for k, h in self.sems.allocated().items():
                sem_by_name[getattr(h, "name", None)] = h
                sem_by_name[str(k)] = h
            for w in waits:
                h = sem_by_name.get(getattr(w, "ant_name", None))
                if h is not None:
                    self.nc.sync.wait_ge(h, w.wait_value)
            self.nc.sync.drain()
            self.nc.all_engine_barrier()
            assert self.sems is not None
            popped = self.nc._tile_sem_poison_stack.pop()
            assert popped is self._sem_poison
            self.nc.clear_and_free_semaphores(list(self.sems.allocated().values()))
            self.nc.all_engine_barrier()

    nc = bass.Bass()
    # per-core inputs (host pre-transposed / pre-cast to bf16)
    xT = nc.declare_dram_parameter("xT", [TOKEN_DIM, N], BF16, isOutput=False)
    wqkvT = nc.declare_dram_parameter("wqkvT", [TOKEN_DIM, 3 * HPC * HEAD_DIM], BF16, isOutput=False)
    woT = nc.declare_dram_parameter("woT", [HPC * HEAD_DIM, TOKEN_DIM], BF16, isOutput=False)
    part = nc.declare_dram_parameter("part", [N, TOKEN_DIM], F32, isOutput=True)

    NT512 = N // 512      # 4
    NT128 = N // 128      # 16
    CCH = TOKEN_DIM // 128  # 8 contraction chunks

    with TC(nc) as tc:
        with (
            tc.tile_pool(name="wsb", bufs=1) as wsb,
            tc.tile_pool(name="qksb", bufs=1) as qksb,
            tc.tile_pool(name="sb", bufs=3) as sb,
            tc.tile_pool(name="ps", bufs=2, space="PSUM") as ps,
        ):
            # ---- load inputs ----
            xT_sb = wsb.tile([128, CCH, N], BF16)      # [c-part, c-chunk, t]
            nc.gpsimd.dma_start(xT_sb[:], xT[:].rearrange("(c p) t -> p c t", p=128))
            wq_sb = wsb.tile([128, CCH, 3 * HPC * HEAD_DIM], BF16)
            nc.gpsimd.dma_start(wq_sb[:], wqkvT[:].rearrange("(c p) r -> p c r", p=128))
            wo_sb = wsb.tile([128, 2, TOKEN_DIM], BF16)  # [i-part, i-chunk, o]
            nc.gpsimd.dma_start(wo_sb[:], woT[:].rearrange("(c p) o -> p c o", p=128))

            # ---- stage 1: QT/KT [2*HPC tiles of [128, N]] and V_aug ----
            qk_tiles = []
            for m in range(2 * HPC * HEAD_DIM // 128):  # 4 tiles: q01,q23,k01,k23
                qk_t = qksb.tile([128, N], BF16, name=f"qk_{m}", tag=f"qk_{m}")
                qk_tiles.append(qk_t)
                for t4 in range(NT512):
                    qkps = ps.tile([128, 512], F32, tag="qkps", bufs=2)
                    for cc in range(CCH):
                        nc.tensor.matmul(
                            qkps[:],
                            wq_sb[:, cc, m * 128:(m + 1) * 128],
                            xT_sb[:, cc, t4 * 512:(t4 + 1) * 512],
                            start=(cc == 0), stop=(cc == CCH - 1),
                        )
                    nc.scalar.copy(qk_t[:, t4 * 512:(t4 + 1) * 512], qkps[:])

            # V natural [t, h, d+1] with ones column
            v_tiles = []
            VR = HPC * HEAD_DIM  # 256
            for t16 in range(NT128):
                v_t = qksb.tile([128, HPC, HEAD_DIM + 1], BF16, name=f"v_{t16}", tag=f"v_{t16}")
                v_tiles.append(v_t)
                vps = ps.tile([128, VR], F32, tag="vps", bufs=1)
                for cc in range(CCH):
                    nc.tensor.matmul(
                        vps[:],
                        xT_sb[:, cc, t16 * 128:(t16 + 1) * 128],
                        wq_sb[:, cc, 2 * VR:3 * VR],
                        start=(cc == 0), stop=(cc == CCH - 1),
                    )
                nc.scalar.copy(
                    v_t[:, :, :HEAD_DIM],
                    vps[:].rearrange("p (h d) -> p h d", h=HPC),
                )
                nc.vector.memset(v_t[:, :, HEAD_DIM:], 1.0)

            # ---- stage 2+3: attention per head, accumulate out-proj ----
            o_all = [qksb.tile([128, N], BF16, name=f"o_{hp}", tag=f"o_{hp}")
                     for hp in range(2)]  # head-pair tiles: rows = 2x64 dims

            for h in range(HPC):
                hp, ho = h // 2, (h % 2) * 64
                kt = qk_tiles[2 + h // 2]   # k tile for this head pair
                qt = qk_tiles[h // 2]
                for t4 in range(NT512):
                    ops_t = ps.tile([65, 512], F32, tag="ops", bufs=1)
                    for s16 in range(NT128):
                        sps = ps.tile([128, 512], F32, tag="sps", bufs=2)
                        nc.tensor.matmul(
                            sps[:],
                            kt[ho:ho + 64, s16 * 128:(s16 + 1) * 128],
                            qt[ho:ho + 64, t4 * 512:(t4 + 1) * 512],
                            start=True, stop=True,
                        )
                        es = sb.tile([128, 512], BF16, tag="es", bufs=4)
                        nc.scalar.activation(es[:], sps[:], AF.Exp)
                        nc.tensor.matmul(
                            ops_t[:],
                            v_tiles[s16][:, h, :],
                            es[:],
                            start=(s16 == 0), stop=(s16 == NT128 - 1),
                        )
                    # normalize: rows 0:64 / row 64
                    rec = sb.tile([1, 512], F32, tag="rec", bufs=2)
                    nc.vector.reciprocal(rec[:], ops_t[64:65, :])
                    rec16 = sb.tile([1, 512], BF16, tag="rec16", bufs=2)
                    nc.scalar.copy(rec16[:], rec[:])
                    ones_sb = sb.tile([1, 64], BF16, tag="ones", bufs=1)
                    nc.vector.memset(ones_sb[:], 1.0)
                    bc_ps = ps.tile([64, 512], F32, tag="bcps", bufs=1)
                    nc.tensor.matmul(bc_ps[:], ones_sb[:], rec16[:],
                                     start=True, stop=True)
                    bc_sb = sb.tile([64, 512], BF16, tag="bcsb", bufs=2)
                    nc.scalar.copy(bc_sb[:], bc_ps[:])
                    nc.vector.tensor_tensor(
                        out=o_all[hp][ho:ho + 64, t4 * 512:(t4 + 1) * 512],
                        in0=ops_t[0:64, :], in1=bc_sb[:],
                        op=mybir.AluOpType.mult,
                    )

            # ---- stage 4: partial out-proj: part[t,o] += O^T[i,t] woT[i,o] ----
            for t16 in range(NT128):
                for o2 in range(2):
                    pps = ps.tile([128, 512], F32, tag="pps", bufs=1)
                    for hp in range(2):
                        nc.tensor.matmul(
                            pps[:],
                            o_all[hp][:, t16 * 128:(t16 + 1) * 128],
                            wo_sb[:, hp, o2 * 512:(o2 + 1) * 512],
                            start=(hp == 0), stop=(hp == 1),
                        )
                    out_sb = sb.tile([128, 512], F32, tag="outsb", bufs=3)
                    nc.scalar.copy(out_sb[:], pps[:])
                    nc.gpsimd.dma_start(
                        part[t16 * 128:(t16 + 1) * 128, o2 * 512:(o2 + 1) * 512],
                        out_sb[:],
                    )
    # this walrus build allows only ONE sync wait per instruction: hoist
    # extra waits onto standalone event-semaphore carriers on the same engine
    nsplit = 0
    for bb in nc.m.functions[0].blocks:
        new_insts = []
        for ins in bb.instructions:
            si = getattr(ins, "sync_info", None)
            if si is not None and len(si.on_wait) > 1:
                waits = list(si.on_wait)
                for w in waits[:-1]:
                    nsplit += 1
                    ev = mybir.InstEventSemaphore(
                        name=f"I-wsplit-{nsplit}", ins=[], outs=[],
                        engine=ins.engine,
                        sync_info=mybir.SyncInfo(on_wait=[w], on_update=[]),
                    )
                    new_insts.append(ev)
                try:
                    si.on_wait.clear()
                    si.on_wait.append(waits[-1])
                except Exception:
                    ins.sync_info = mybir.SyncInfo(
                        on_wait=[waits[-1]], on_update=list(si.on_update)
                    )
            new_insts.append(ins)
        bb.instructions = new_insts
    return nc


def kernel(x, w_qkv, w_out, b_out):
    from concourse.bass_utils import run_bass_kernel_spmd

    if "nc" not in _cache:
        _cache["nc"] = _build()
    nc = _cache["nc"]

    bf = ml_dtypes.bfloat16
    scale = HEAD_DIM ** -0.5
    x = np.asarray(x)
    w_qkv = np.asarray(w_qkv)
    w_out = np.asarray(w_out)
    b_out = np.asarray(b_out)

    in_maps = []
    for c in range(CORES):
        g, j = c // 4, c % 4
        hsl = slice(j * HPC * HEAD_DIM, (j + 1) * HPC * HEAD_DIM)
        wq = w_qkv[0 * INNER:1 * INNER][hsl] * scale   # fold softmax scale into Q
        wk = w_qkv[1 * INNER:2 * INNER][hsl]
        wv = w_qkv[2 * INNER:3 * INNER][hsl]
        wqkvT = np.concatenate([wq, wk, wv], 0).T.astype(bf)  # [1024, 768]
        woT = w_out[:, hsl].T.astype(bf)                      # [256, 1024]
        in_maps.append({
            "xT": np.ascontiguousarray(x[g].T).astype(bf),
            "wqkvT": np.ascontiguousarray(wqkvT),
            "woT": np.ascontiguousarray(woT),
        })

    res = run_bass_kernel_spmd(nc, in_maps, list(range(CORES)))
    _cache["last_res"] = res
    out = np.empty((B, N, TOKEN_DIM), dtype=np.float32)
    for g in range(GROUPS):
        acc = res.results[4 * g]["part"].astype(np.float32).copy()
        for j in range(1, 4):
            acc += res.results[4 * g + j]["part"]
        out[g] = acc + b_out[None, :]
    return out

